# revision 3
# baseline (speedup 1.0000x reference)
"""Trainium2 Bass kernel for nn_Chambers: 6 per-chamber MLPs over a shared
reservoir input, followed by 5 coupled-chamber fixed-point iterations.

Data-parallel over 8 NeuronCores: each core processes B/8 = 32768 rows.

v3 design (ACT-engine-bound; trace-driven rework of v2):
  - PSUM pool is 2 x [128,2048] f32 (4 banks each = whole PSUM): every MLP
    silu is ONE ACT op per chamber (L1), per pair (L2), per group (L3)
    covering a full 2048-row chunk -> 11 ACT ops/chunk instead of 21.
  - L4 accumulates the whole chunk's [6,2048] raw strip in one PSUM tile,
    one DVE evac copy (bf16), DMA-scattered into batch-major rawbm.
  - Coupling is rewritten from 24 1x-mode STT pair-AXPYs per iter to a
    cyclic-rotation form: sin ACT writes [S|C] into blocks {0,2} of a
    [128,24F] tile, one strided DVE copy duplicates to blocks {1,3}
    ([S|S'|C|C']), then M = [KC.S | KC.C] is 5 pattern-tile TT mults +
    4 TT adds (all bf16 2x mode), U = M * [C|S], DD = U1-U2,
    A = clip(D+DD). ~15 DVE ops/iter, all contiguous-inner bf16.
  - Coupling state and outputs are bf16 (tolerance 2e-2; measured ~5e-3).
  - Slices (16384, 8192, 4096, 2048, 2048): big slices early amortize
    coupling dispatch overhead while later MLP hides them; the last slice
    is minimal so the un-overlapped final coupling tail is short.
"""

import numpy as np

# ---- problem constants (fixed by the task; kernel.py must be self-contained)
B = 262144
RES_DIM = 100
NCH = 6
CF_ITERS = 5
CF_K = 0.02
DECAY = np.array([0.9, 0.93, 0.85, 0.97, 0.88, 0.94], dtype=np.float32)
COUPLING = np.array([
    [0.0, -0.3, 0.6, 0.4, -0.2, 0.3],
    [-0.3, 0.0, -0.5, -0.7, 0.6, 0.4],
    [0.6, -0.5, 0.0, 0.3, -0.3, 0.2],
    [0.4, -0.7, 0.3, 0.0, -0.4, 0.5],
    [-0.2, 0.6, -0.3, -0.4, 0.0, 0.3],
    [0.3, 0.4, 0.2, 0.5, 0.3, 0.0]], dtype=np.float32)
N_CORES = 8
R_CORE = B // N_CORES          # 32768 rows per core
CHUNK = 2048                   # rows per MLP chunk
HALF_PI = float(np.pi / 2.0)
DEFAULT_SLICES = (16384, 8192, 4096, 2048, 2048)
FMAX = 128
ACT_3D_DEST = True             # sin ACT writes blocks {0,2} via 3D dest AP

_BUILD_CACHE = {}


def _pattern_offsets(slice_sizes):
    fs = sorted({s // 128 for s in slice_sizes}, reverse=True)
    f_off = {}
    off = 0
    for F in fs:
        f_off[F] = off
        off += 6 * F
    k_off = {}
    koff = 0
    for F in fs:
        k_off[F] = koff
        koff += 60 * F          # 5 k-blocks x [128, 12F]
    return fs, f_off, off, k_off, koff


def _build(R, slice_sizes):
    """Emit + compile the per-core SPMD program."""
    from contextlib import ExitStack
    import concourse.bass as bass
    import concourse.mybir as mybir
    from concourse import bacc, tile

    f32 = mybir.dt.float32
    bf16 = mybir.dt.bfloat16
    AF = mybir.ActivationFunctionType
    OP = mybir.AluOpType

    assert sum(slice_sizes) == R and all(s % CHUNK == 0 for s in slice_sizes)
    assert all(CHUNK % (s // 128) == 0 for s in slice_sizes)
    fs, f_off, PATW, k_off, KPW = _pattern_offsets(slice_sizes)

    nc = bacc.Bacc("TRN2", target_bir_lowering=False, debug=False,
                   num_devices=N_CORES)
    res = nc.dram_tensor("res_t", [RES_DIM, R], bf16, kind="ExternalInput").ap()
    w1t = nc.dram_tensor("w1t", [RES_DIM, 6 * 128], bf16, kind="ExternalInput").ap()
    b1t = nc.dram_tensor("b1t", [128, 6], f32, kind="ExternalInput").ap()
    w2t = nc.dram_tensor("w2t", [128, 6 * 64], bf16, kind="ExternalInput").ap()
    b2p = nc.dram_tensor("b2p", [128, 3], f32, kind="ExternalInput").ap()
    # w3t holds W3^T twice (rows 0-63 and 64-127) for row-tiled matmuls
    w3t = nc.dram_tensor("w3t", [128, 6 * 32], bf16, kind="ExternalInput").ap()
    b3x = nc.dram_tensor("b3x", [128, 1], f32, kind="ExternalInput").ap()
    b3y = nc.dram_tensor("b3y", [128, 1], f32, kind="ExternalInput").ap()
    w4a = nc.dram_tensor("w4a", [128, 6], bf16, kind="ExternalInput").ap()
    w4b = nc.dram_tensor("w4b", [128, 6], bf16, kind="ExternalInput").ap()
    decp = nc.dram_tensor("decp", [128, PATW], f32, kind="ExternalInput").ap()
    b4p = nc.dram_tensor("b4p", [128, PATW], f32, kind="ExternalInput").ap()
    kp2 = nc.dram_tensor("kp2", [128, KPW], bf16, kind="ExternalInput").ap()
    act_o = nc.dram_tensor("act_o", [6, R], bf16, kind="ExternalOutput").ap()
    raw_o = nc.dram_tensor("raw_o", [6, R], bf16, kind="ExternalOutput").ap()

    H = CHUNK // 2

    def emit():
        with tile.TileContext(nc) as tc, ExitStack() as ctx:
            wp = ctx.enter_context(tc.tile_pool(name="w", bufs=1))
            # critical-path weights (L1 needs these first) on the sync
            # HWDGE queue; the rest on gpsimd; coupling patterns (first
            # used ~100us in) last
            t_w1t = wp.tile([RES_DIM, 6 * 128], bf16, tag="w1t")
            nc.sync.dma_start(t_w1t[:], w1t)
            t_b1t = wp.tile([128, 6], f32, tag="b1t")
            nc.sync.dma_start(t_b1t[:], b1t)
            t_w2t = wp.tile([128, 6 * 64], bf16, tag="w2t")
            nc.gpsimd.dma_start(t_w2t[:], w2t)
            t_b2p = wp.tile([128, 3], f32, tag="b2p")
            nc.gpsimd.dma_start(t_b2p[:], b2p)
            t_w3t = wp.tile([128, 6 * 32], bf16, tag="w3t")
            nc.gpsimd.dma_start(t_w3t[:], w3t)
            t_b3x = wp.tile([128, 1], f32, tag="b3x")
            nc.gpsimd.dma_start(t_b3x[:], b3x)
            t_b3y = wp.tile([128, 1], f32, tag="b3y")
            nc.gpsimd.dma_start(t_b3y[:], b3y)
            t_w4a = wp.tile([128, 6], bf16, tag="w4a")
            nc.gpsimd.dma_start(t_w4a[:], w4a)
            t_w4b = wp.tile([128, 6], bf16, tag="w4b")
            nc.gpsimd.dma_start(t_w4b[:], w4b)
            t_decp = wp.tile([128, PATW], f32, tag="decp")
            nc.gpsimd.dma_start(t_decp[:], decp)
            t_b4p = wp.tile([128, PATW], f32, tag="b4p")
            nc.gpsimd.dma_start(t_b4p[:], b4p)
            t_kp2 = wp.tile([128, KPW], bf16, tag="kp2")
            nc.gpsimd.dma_start(t_kp2[:], kp2)

            p_rT = ctx.enter_context(tc.tile_pool(name="rT", bufs=3))
            p_mm = ctx.enter_context(tc.tile_pool(name="pmm", bufs=2, space="PSUM"))
            p_h1 = ctx.enter_context(tc.tile_pool(name="h1", bufs=3))
            p_h2 = ctx.enter_context(tc.tile_pool(name="h2", bufs=2))
            p_h3 = ctx.enter_context(tc.tile_pool(name="h3", bufs=2))
            p_rsb = ctx.enter_context(tc.tile_pool(name="rsb", bufs=2))
            p_bm = ctx.enter_context(tc.tile_pool(name="bm", bufs=2))
            p_cpl = ctx.enter_context(tc.tile_pool(name="cpl", bufs=2))
            p_scr = ctx.enter_context(tc.tile_pool(name="scr", bufs=1))

            g0 = 0
            for s, srows in enumerate(slice_sizes):
                F = srows // 128
                PPC = CHUNK // F
                SIX = 6 * F
                po = f_off[F]
                ko = k_off[F]
                chunks_per_slice = srows // CHUNK
                rawbm = p_bm.tile([128, 6 * FMAX], bf16, tag="rawbm")
                for k in range(chunks_per_slice):
                    g = g0 + k                         # global chunk id
                    rT = p_rT.tile([RES_DIM, CHUNK], bf16, tag="rT")
                    nc.sync.dma_start(rT[:], res[:, g * CHUNK:(g + 1) * CHUNK])

                    # -- L1 (+ L2 when a pair completes); [128,2048] PSUM
                    # tiles, one silu per chamber / per pair
                    h2 = p_h2.tile([128, 3 * CHUNK], bf16, tag="h2")
                    h1p = None
                    for c in range(6):
                        if c % 2 == 0:
                            h1p = p_h1.tile([128, 2 * CHUNK], bf16, tag="h1p")
                        ps = p_mm.tile([128, CHUNK], f32, tag="mm")
                        for q in range(4):
                            o = q * 512
                            nc.tensor.matmul(
                                ps[:, o:o + 512],
                                t_w1t[:, c * 128:(c + 1) * 128],
                                rT[:, o:o + 512])
                        nc.scalar.activation(
                            h1p[:, (c % 2) * CHUNK:(c % 2 + 1) * CHUNK],
                            ps[:], AF.Silu, bias=t_b1t[:, c:c + 1])
                        if c % 2 == 1:
                            p = c // 2
                            ps2 = p_mm.tile([128, CHUNK], f32, tag="mm")
                            for q in range(4):
                                o = q * 512
                                nc.tensor.matmul(
                                    ps2[0:64, o:o + 512],
                                    t_w2t[:, (2 * p) * 64:(2 * p + 1) * 64],
                                    h1p[:, o:o + 512],
                                    tile_position=(0, 0))
                                nc.tensor.matmul(
                                    ps2[64:128, o:o + 512],
                                    t_w2t[:, (2 * p + 1) * 64:(2 * p + 2) * 64],
                                    h1p[:, CHUNK + o:CHUNK + o + 512],
                                    tile_position=(0, 64))
                            nc.scalar.activation(
                                h2[:, p * CHUNK:(p + 1) * CHUNK],
                                ps2[:], AF.Silu, bias=t_b2p[:, p:p + 1])

                    # -- L3 X: chambers 0-3 -> [128, 2048], one silu
                    h3 = p_h3.tile([128, 3 * H], bf16, tag="h3")
                    psx = p_mm.tile([128, CHUNK], f32, tag="mm")
                    for q in range(4):
                        o = q * 512
                        for c in range(4):
                            p = c // 2
                            half = c % 2  # which 64-row half of the pair tile
                            nc.tensor.matmul(
                                psx[32 * c:32 * (c + 1), o:o + 512],
                                t_w3t[64 * half:64 * half + 64,
                                      c * 32:(c + 1) * 32],
                                h2[64 * half:64 * half + 64,
                                   p * CHUNK + o:p * CHUNK + o + 512],
                                tile_position=(64 * half, 32 * c))
                    nc.scalar.activation(h3[:, 0:CHUNK], psx[:],
                                         AF.Silu, bias=t_b3x[:])

                    # -- L3 Y: c4/c5, chunk halves stacked in partitions:
                    # [0:32]=c4 cols 0:H, [32:64]=c5 cols 0:H,
                    # [64:96]=c4 cols H:2H, [96:128]=c5 cols H:2H
                    psy = p_mm.tile([128, CHUNK], f32, tag="mm")
                    for hh in range(2):                 # chunk half
                        for ci, c in enumerate((4, 5)):
                            half = ci               # c4 lower, c5 upper rows
                            pb = 64 * hh + 32 * ci
                            for q in range(2):
                                o = hh * H + q * 512
                                nc.tensor.matmul(
                                    psy[pb:pb + 32, q * 512:(q + 1) * 512],
                                    t_w3t[64 * half:64 * half + 64,
                                          c * 32:(c + 1) * 32],
                                    h2[64 * half:64 * half + 64,
                                       2 * CHUNK + o:2 * CHUNK + o + 512],
                                    tile_position=(64 * half, pb))
                    nc.scalar.activation(h3[:, CHUNK:CHUNK + H], psy[:, 0:H],
                                         AF.Silu, bias=t_b3y[:])

                    # -- L4: raw[0:6] accumulating K-stacked matmuls into a
                    # single [6, 2048] strip
                    ps4 = p_mm.tile([128, CHUNK], f32, tag="mm")
                    for q4 in range(4):
                        hh = q4 // 2
                        qq = q4 % 2
                        o = q4 * 512
                        nc.tensor.matmul(
                            ps4[0:6, o:o + 512], t_w4a[:, 0:6],
                            h3[:, o:o + 512],
                            start=True, stop=False, tile_position=(0, 0))
                        # c4/c5 features: partitions 64*hh:64*hh+64 of h3y
                        nc.tensor.matmul(
                            ps4[0:6, o:o + 512],
                            t_w4b[64 * hh:64 * hh + 64, 0:6],
                            h3[64 * hh:64 * hh + 64,
                               CHUNK + qq * 512:CHUNK + (qq + 1) * 512],
                            start=False, stop=True,
                            tile_position=(64 * hh, 0))
                    rsb = p_rsb.tile([6, CHUNK], bf16, tag="rsb")
                    nc.vector.tensor_copy(rsb[:], ps4[0:6, :])
                    # scatter into batch-major rawbm
                    for c in range(6):
                        nc.gpsimd.dma_start(
                            rawbm[k * PPC:(k + 1) * PPC, c * F:(c + 1) * F],
                            rsb[c:c + 1, :].rearrange("o (a f) -> o a f", f=F))

                # ---- coupling for slice s (batch-major [128, 6F])
                # raw outputs don't depend on coupling: emit first so the
                # DMAs overlap the coupling iterations
                r0 = g0 * CHUNK
                for c in range(6):
                    q = nc.sync if c % 2 == 0 else nc.gpsimd
                    q.dma_start(
                        raw_o[c:c + 1, r0:r0 + srows]
                        .rearrange("o (p f) -> (o p) f", f=F),
                        rawbm[:, c * F:(c + 1) * F])

                def ctile(pool, w, dt, tag):
                    t = pool.tile([128, w * FMAX], dt, tag=tag)
                    return t[:, 0:w * F]

                rawb = ctile(p_cpl, 6, f32, "RB")
                nc.vector.tensor_tensor(rawb, rawbm[:, 0:SIX],
                                        t_b4p[:, po:po + SIX], OP.add)
                tt6 = ctile(p_scr, 6, bf16, "T6")
                nc.scalar.activation(tt6, rawb, AF.Tanh, scale=0.5)
                A = ctile(p_cpl, 6, bf16, "A")
                nc.vector.tensor_scalar(A, tt6, 0.5, 0.5, OP.mult, OP.add)
                for it in range(CF_ITERS):
                    D2 = ctile(p_scr, 12, f32, "D2")
                    D = D2[:, 0:SIX]
                    nc.vector.tensor_tensor(D, A, t_decp[:, po:po + SIX],
                                            OP.mult)
                    nc.vector.tensor_scalar(D2[:, SIX:2 * SIX], D,
                                            HALF_PI, None, OP.add)
                    # SCSC layout: [S | S' | C | C'], each block 6F wide
                    SCSC = ctile(p_scr, 24, bf16, "SC")
                    sc4 = SCSC.rearrange("p (u x) -> p u x", u=4)
                    if ACT_3D_DEST:
                        nc.scalar.activation(
                            sc4[:, 0::2, :],
                            D2.rearrange("p (u x) -> p u x", u=2), AF.Sin)
                    else:
                        nc.scalar.activation(SCSC[:, 0:SIX], D2[:, 0:SIX],
                                             AF.Sin)
                        nc.scalar.activation(SCSC[:, 2 * SIX:3 * SIX],
                                             D2[:, SIX:2 * SIX], AF.Sin)
                    nc.vector.tensor_copy(sc4[:, 1::2, :], sc4[:, 0::2, :])
                    # M = [KC.S | KC.C]: 5 rotation products + tree adds
                    sc2 = SCSC.rearrange("p (u x) -> p u x", u=2)
                    Pt = []
                    for kk in range(1, 6):
                        P = ctile(p_scr, 12, bf16, f"P{kk}")
                        nc.vector.tensor_tensor(
                            P.rearrange("p (u x) -> p u x", u=2),
                            t_kp2[:, ko + (kk - 1) * 2 * SIX:
                                  ko + kk * 2 * SIX]
                            .rearrange("p (u x) -> p u x", u=2),
                            sc2[:, :, kk * F:kk * F + SIX], OP.mult)
                        Pt.append(P)
                    nc.vector.tensor_tensor(Pt[0], Pt[0], Pt[1], OP.add)
                    nc.vector.tensor_tensor(Pt[2], Pt[2], Pt[3], OP.add)
                    nc.vector.tensor_tensor(Pt[0], Pt[0], Pt[4], OP.add)
                    nc.vector.tensor_tensor(Pt[0], Pt[0], Pt[2], OP.add)
                    M = Pt[0]
                    # U = M * [C | S]  (blocks {2, 0})
                    U = ctile(p_scr, 12, bf16, "U")
                    nc.vector.tensor_tensor(
                        U.rearrange("p (u x) -> p u x", u=2),
                        M.rearrange("p (u x) -> p u x", u=2),
                        sc4[:, 2::-2, :], OP.mult)
                    DD = ctile(p_scr, 6, bf16, "DD")
                    nc.vector.tensor_tensor(DD, U[:, 0:SIX], U[:, SIX:2 * SIX],
                                            OP.subtract)
                    Vt = ctile(p_scr, 6, bf16, "V")
                    nc.vector.tensor_tensor(Vt, D, DD, OP.add)
                    A = ctile(p_cpl, 6, bf16, "A")
                    nc.vector.tensor_scalar(A, Vt, 0.0, 1.0, OP.max, OP.min)

                # ---- act outputs: contiguous per-chamber DMAs
                # (row = r0 + p*F + f), split across two trigger queues
                for c in range(6):
                    q = nc.sync if c % 2 == 0 else nc.gpsimd
                    q.dma_start(
                        act_o[c:c + 1, r0:r0 + srows]
                        .rearrange("o (p f) -> (o p) f", f=F),
                        A[:, c * F:(c + 1) * F])
                g0 += chunks_per_slice
    return nc, emit


def prep_weights(W1, b1, W2, b2, W3, b3, W4, b4, slice_sizes):
    """Host-side weight layout preparation."""
    import ml_dtypes
    bf16 = ml_dtypes.bfloat16
    d = {}
    d["w1t"] = np.ascontiguousarray(
        W1.transpose(2, 0, 1).reshape(RES_DIM, 6 * 128)).astype(bf16)
    d["b1t"] = np.ascontiguousarray(b1.T)                      # [128, 6]
    d["w2t"] = np.ascontiguousarray(
        W2.transpose(2, 0, 1).reshape(128, 6 * 64)).astype(bf16)
    b2p = np.zeros((128, 3), np.float32)
    for p in range(3):
        b2p[0:64, p] = b2[2 * p]
        b2p[64:128, p] = b2[2 * p + 1]
    d["b2p"] = b2p
    w3t_h = W3.transpose(2, 0, 1).reshape(64, 6 * 32)
    d["w3t"] = np.ascontiguousarray(
        np.concatenate([w3t_h, w3t_h], axis=0)).astype(bf16)
    b3x = np.zeros((128, 1), np.float32)
    for c in range(4):
        b3x[32 * c:32 * (c + 1), 0] = b3[c]
    d["b3x"] = b3x
    b3y = np.zeros((128, 1), np.float32)
    b3y[0:32, 0] = b3[4]
    b3y[32:64, 0] = b3[5]
    b3y[64:96, 0] = b3[4]
    b3y[96:128, 0] = b3[5]
    d["b3y"] = b3y
    w4a = np.zeros((128, 6), np.float32)
    for c in range(4):
        w4a[32 * c:32 * (c + 1), c] = W4[c, 0, :]
    d["w4a"] = w4a.astype(bf16)
    w4b = np.zeros((128, 6), np.float32)
    w4b[0:32, 4] = W4[4, 0, :]
    w4b[32:64, 5] = W4[5, 0, :]
    w4b[64:96, 4] = W4[4, 0, :]
    w4b[96:128, 5] = W4[5, 0, :]
    d["w4b"] = w4b.astype(bf16)
    fs, f_off, patw, k_off, kpw = _pattern_offsets(slice_sizes)
    decp = np.zeros((128, patw), np.float32)
    b4pp = np.zeros((128, patw), np.float32)
    for F in fs:
        off = f_off[F]
        for c in range(6):
            decp[:, off + c * F:off + (c + 1) * F] = DECAY[c]
            b4pp[:, off + c * F:off + (c + 1) * F] = b4[c, 0]
    d["decp"] = decp
    d["b4p"] = b4pp
    KC = (CF_K * COUPLING).astype(np.float32)
    kp = np.zeros((128, kpw), np.float32)
    for F in fs:
        ko = k_off[F]
        for kk in range(1, 6):
            base = ko + (kk - 1) * 12 * F
            for u in range(2):
                for i in range(6):
                    col = base + (u * 6 + i) * F
                    kp[:, col:col + F] = KC[i][(i + kk) % 6]
    d["kp2"] = kp.astype(bf16)
    d["_b4"] = np.ascontiguousarray(b4[:, 0])                  # host-only
    return d


def build_program(R=R_CORE, slice_sizes=DEFAULT_SLICES):
    """Build + bacc-compile the program (cached)."""
    key = (R, tuple(slice_sizes))
    if key in _BUILD_CACHE:
        return _BUILD_CACHE[key]
    nc, emit = _build(R, list(slice_sizes))
    emit()
    nc.compile()
    _BUILD_CACHE[key] = nc
    return nc


def kernel(res, W1, b1, W2, b2, W3, b3, W4, b4, coupling):
    """Full-input entry point: shards res over 8 cores, runs the SPMD
    kernel, gathers and returns (act, raw) like the reference."""
    from concourse.bass_utils import run_bass_kernel_spmd

    res = np.ascontiguousarray(np.asarray(res, np.float32))
    W1 = np.asarray(W1, np.float32); b1 = np.asarray(b1, np.float32)
    W2 = np.asarray(W2, np.float32); b2 = np.asarray(b2, np.float32)
    W3 = np.asarray(W3, np.float32); b3 = np.asarray(b3, np.float32)
    W4 = np.asarray(W4, np.float32); b4 = np.asarray(b4, np.float32)

    wd = prep_weights(W1, b1, W2, b2, W3, b3, W4, b4, DEFAULT_SLICES)
    b4vec = wd.pop("_b4")
    nc = build_program(R_CORE)

    import ml_dtypes
    res_t = np.ascontiguousarray(res.T.astype(ml_dtypes.bfloat16))  # [100, B]
    in_maps = []
    for i in range(N_CORES):
        m = dict(wd)
        m["res_t"] = np.ascontiguousarray(res_t[:, i * R_CORE:(i + 1) * R_CORE])
        in_maps.append(m)
    out = run_bass_kernel_spmd(nc, in_maps, list(range(N_CORES)))
    act = np.concatenate(
        [np.ascontiguousarray(
            out.results[i]["act_o"].astype(np.float32).T)
         for i in range(N_CORES)], axis=0)
    raw = np.concatenate(
        [np.ascontiguousarray(
            out.results[i]["raw_o"].astype(np.float32).T)
         for i in range(N_CORES)], axis=0)
    raw = raw + b4vec[None, :]
    return act.astype(np.float32), raw.astype(np.float32)
